# revision 33
# baseline (speedup 1.0000x reference)
"""BIDE forward kernel for Trainium2, 8-core data parallel over B — v5.2.

Math (per batch row; 2 rows per core):
  logit(v) = sum_h cos(2 pi z[h,v]),  z = W' . bits(v) + r'  (W' = W/2pi)
  out[t] = logit(x_t) - logZ,  logZ = 60 + ln sum_v exp(logit(v) - 60)

All trig goes through ONE spline (Sin, domain [-pi, pi]) via the
Chebyshev double-double-angle form
    cos(2 pi q) = 1 + 8 d,   d = (v - 1) v,   v = sin^2(pi q / 2)
exact for |q| <= 2 and invariant under integer shifts of q.

logZ path: z splits into zlo (low 8 bits, + r') and zhi (high 8 bits),
each enumerated over 256 values; |z| <= 0.93 < 1 (verified from W), so
s = sin(pi z/2) and c = sin(pi z/2 + pi/2) need NO range reduction.
DVE builds chat = cos(2 pi z) = 1+8d and e8 = (8v-4)sc = 8(v-0.5)sc
(hi-half weights negated on host, so e8_hi = -8 e_hi while chat is
unaffected); the 256x256 logit table is then just two K=128 matmuls
per 128-row half:
    table = chat_hi^T chat_lo + e8_hi^T e8_lo
exp(table - 60) with accum_out gives S_b per partition.

logit_x path: q[h,t] = W' bits(x_t) + frac(r') via a K=17 matmul
(|q| <= 1.27 for every attainable pattern), Sin straight from PSUM,
v = s*s, vm1 = v-1, d = v*vm1 on DVE, and an h-fold matmul whose
one-hot weights carry the 8x: hs = 8 sum_h d.  out = hs + (68 - lnS).

Scheduling (all HW-measured this session):
 - PE runs at 1.2 GHz on this instance (a 3.6us back-to-back matmul
   warmup never lifted the HAM clock gate), so matmul columns are the
   budget; the table product is reduced to 2 MMs/half and emission
   interleaves x-path matmuls between table groups to avoid PE stalls.
 - ACT queue must be [all Sins][Exp Exp][Ln] for minimal table-set
   loads; the Exp bias tile is produced by an ACT Identity that READS
   the last x-sin output, so the scheduler topologically cannot hoist
   Exp between Sins. get_activation_tables is patched so Exp and Ln
   share natural_log_exp_and_others (one load).
 - The final output add runs on ACT (Identity + per-partition bias),
   off the busy DVE.
 - PSUM (8 banks): qz+x-unit pool 2x[128,1024] (4) + table products 2
   + h-fold accumulator 1 + logZ scalars 1.

Dead ends (HW-measured): indirect-DMA gathers ~4.5ns/desc serialized;
gpsimd ap_gather ~27us/512; GPSIMD tensor_scalar 7-15us AND stalls
concurrent DVE ops; GPSIMD cannot read PSUM, no tensor_tensor codegen;
AluOpType.mod rejected; ACT Sin garbage outside [-pi,pi]; re-opening a
closed PSUM accumulation group overwrites instead of accumulating;
scalar_tensor_tensor runs at 1x on DVE.
"""

import numpy as np
import ml_dtypes
from contextlib import ExitStack

import concourse.bacc as bacc
import concourse.bass as bass
from concourse import mybir
from concourse.bass_utils import run_bass_kernel_spmd
from concourse.tile import TileContext

F32 = mybir.dt.float32
BF16 = mybir.dt.bfloat16

PI_2 = float(np.float32(np.pi / 2.0))
INV_2PI = 1.0 / (2.0 * np.pi)
# logits peak ~89: shift exp so it stays in fp32 / the ACT Ln spline range
EXP_SHIFT = 60.0

N_CORES = 8
B, H, T = 16, 128, 4096
BPC = B // N_CORES  # batch rows per core (2)


def _patch_act_tables(arch: str):
    """Force Exp and Ln to resolve to natural_log_exp_and_others so the
    [sins][exp][ln] queue needs one nl-exp set load, not two."""
    from concourse.hw_specs import get_activation_tables

    tabs = get_activation_tables(arch)
    for name, fns in tabs.items():
        if name != "natural_log_exp_and_others":
            fns.discard(mybir.ActivationFunctionType.Exp)
            fns.discard(mybir.ActivationFunctionType.Ln)


def _build():
    nc = bacc.Bacc("TRN2", target_bir_lowering=False, debug=False)
    _patch_act_tables(nc.m.arch)

    # packed weights: cols [0:512) table z blocks [9,128] at 128*(2b+half)
    # (rows 0-7 bit weights — hi half negated; row 8 = r' on lo, 0 on hi),
    # [1024:1280) bit-plane enumeration bits9, [1280:1536) wx (x-path;
    # rows 0-15 W'.T, 16 frac(r'); block b at 1280+128b)
    wmeta = nc.dram_tensor("wmeta", [17, 1536], BF16, kind="ExternalInput")
    # bit-planes of x: row n = bit_n(x[b, t]), row 16 = 1; col 4096b + t
    bitsx = nc.dram_tensor("bitsx", [17, 8192], BF16, kind="ExternalInput")
    # h-sum one-hot columns scaled by 8: aux[h, 16vg+m] = 8*(m==vg)
    aux = nc.dram_tensor("aux", [128, 256], BF16, kind="ExternalInput")
    # negsel[k, m] = -1 if m//8 == k else 0 (broadcasts -ln S_b)
    negsel_in = nc.dram_tensor("negsel", [2, 16], F32, kind="ExternalInput")
    # row vg = 8b + 2u + h2 covers t in [1024u + 512h2, +512) of batch b
    out = nc.dram_tensor("out", [16, 512], F32, kind="ExternalOutput")

    with ExitStack() as ctx:
        tc = ctx.enter_context(TileContext(nc))
        sb = ctx.enter_context(tc.tile_pool(name="sb", bufs=1))
        px = ctx.enter_context(tc.tile_pool(name="px", bufs=2, space="PSUM"))
        pst = ctx.enter_context(tc.tile_pool(name="pst", bufs=2, space="PSUM"))
        psh = ctx.enter_context(tc.tile_pool(name="psh", bufs=1, space="PSUM"))
        pss = ctx.enter_context(tc.tile_pool(name="pss", bufs=1, space="PSUM"))

        # ---- input loads on two parallel HWDGE queues
        wmeta_sb = sb.tile([17, 1536], BF16, tag="wmeta")
        bitsx_sb = sb.tile([17, 8192], BF16, tag="bitsx")
        aux_sb = sb.tile([128, 256], BF16, tag="aux")
        negsel = sb.tile([2, 16], F32, tag="negsel")
        # bitsx rides the sync queue alone and wmeta leads the scalar
        # queue: each path's first semaphore fires earliest, unblocking
        # the x-unit matmuls (bitsx) and table matmuls (wmeta) in parallel
        nc.sync.dma_start(out=bitsx_sb[:, 0:4096], in_=bitsx[:, 0:4096])
        nc.sync.dma_start(out=bitsx_sb[:, 4096:8192], in_=bitsx[:, 4096:8192])
        nc.scalar.dma_start(out=wmeta_sb[:], in_=wmeta[:])
        nc.scalar.dma_start(out=aux_sb[:], in_=aux[:])
        nc.scalar.dma_start(out=negsel[:], in_=negsel_in[:])

        # ---- constants
        ones = sb.tile([128, 1], F32, tag="ones")
        nc.vector.memset(ones[:], 1.0)
        bpi2 = sb.tile([128, 1], F32, tag="bpi2")
        nc.vector.memset(bpi2[:], PI_2)
        bm60 = sb.tile([128, 1], F32, tag="bm60")
        nc.vector.memset(bm60[:], -EXP_SHIFT)

        # ---- table path: z enumerations in pst-pool tiles (the tb slots
        # are free this early), keeping the px pool exclusively for x
        # units so u1 need not wait for the table sins
        qzs = []
        for b in range(BPC):
            qzb = pst.tile([128, 512], F32, tag="tb")
            qzs.append(qzb)
            for half in range(2):
                cs = 128 * (2 * b + half)
                nc.tensor.matmul(
                    out=qzb[:, 256 * half : 256 * half + 256],
                    lhsT=wmeta_sb[0:9, cs : cs + 128],
                    rhs=wmeta_sb[0:9, 1024:1280],
                    start=True, stop=True,
                )

        # ---- x path part 1: first two units' matmuls (emitted early so
        # the PE queue has work while the table's DVE chain runs)
        def qx_unit(u):
            b, cu = divmod(u, 4)
            col = 4096 * b + 1024 * cu
            qx = px.tile([128, 1024], F32, tag="u")
            for h2 in range(2):
                nc.tensor.matmul(
                    out=qx[:, 512 * h2 : 512 * h2 + 512],
                    lhsT=wmeta_sb[0:17, 1280 + 128 * b : 1408 + 128 * b],
                    rhs=bitsx_sb[0:17, col + 512 * h2 : col + 512 * h2 + 512],
                    start=True, stop=True,
                )
            s = sb.tile([128, 1024], BF16, tag=f"s{u}")
            nc.scalar.activation(
                out=s[:], in_=qx[:],
                func=mybir.ActivationFunctionType.Sin, scale=PI_2,
            )
            v = sb.tile([128, 1024], BF16, tag=f"v{u}")
            nc.vector.tensor_tensor(
                out=v[:], in0=s[:], in1=s[:], op=mybir.AluOpType.mult,
            )
            vm1 = sb.tile([128, 1024], BF16, tag=f"vm{u}")
            nc.vector.tensor_scalar(
                out=vm1[:], in0=v[:], scalar1=1.0, scalar2=None,
                op0=mybir.AluOpType.subtract,
            )
            d = sb.tile([128, 1024], BF16, tag=f"d{u}")
            nc.vector.tensor_tensor(
                out=d[:], in0=v[:], in1=vm1[:], op=mybir.AluOpType.mult,
            )
            return s, d

        def hfold(u, d, hs_ps):
            for h2 in range(2):
                vg = 2 * u + h2
                nc.tensor.matmul(
                    out=hs_ps[:],
                    lhsT=aux_sb[:, 16 * vg : 16 * vg + 16],
                    rhs=d[:, 512 * h2 : 512 * h2 + 512],
                    start=(vg == 0), stop=(vg == 15),
                )

        # table trig + DVE chain per b -> chat (=cos 2 pi z) and e8
        def table_chain(b):
            s = sb.tile([128, 512], BF16, tag=f"ts{b}")
            nc.scalar.activation(
                out=s[:], in_=qzs[b][:],
                func=mybir.ActivationFunctionType.Sin, scale=PI_2,
            )
            c = sb.tile([128, 512], BF16, tag=f"tc{b}")
            nc.scalar.activation(
                out=c[:], in_=qzs[b][:],
                func=mybir.ActivationFunctionType.Sin, scale=PI_2,
                bias=bpi2[:],
            )
            v = sb.tile([128, 512], BF16, tag=f"tv{b}")
            nc.vector.tensor_tensor(
                out=v[:], in0=s[:], in1=s[:], op=mybir.AluOpType.mult,
            )
            sc = sb.tile([128, 512], BF16, tag=f"tsc{b}")
            nc.vector.tensor_tensor(
                out=sc[:], in0=s[:], in1=c[:], op=mybir.AluOpType.mult,
            )
            # chat = (8v-8)v + 1 = cos(2 pi z); e8 = (8v-4)sc
            v8m8 = sb.tile([128, 512], BF16, tag=f"t88{b}")
            nc.vector.tensor_scalar(
                out=v8m8[:], in0=v[:], scalar1=8.0, scalar2=8.0,
                op0=mybir.AluOpType.mult, op1=mybir.AluOpType.subtract,
            )
            cpre = sb.tile([128, 512], BF16, tag=f"tcp{b}")
            nc.vector.tensor_tensor(
                out=cpre[:], in0=v8m8[:], in1=v[:], op=mybir.AluOpType.mult,
            )
            chat = sb.tile([128, 512], BF16, tag=f"tch{b}")
            nc.vector.tensor_scalar(
                out=chat[:], in0=cpre[:], scalar1=1.0, scalar2=None,
                op0=mybir.AluOpType.add,
            )
            t8 = sb.tile([128, 512], BF16, tag=f"tt8{b}")
            nc.vector.tensor_scalar(
                out=t8[:], in0=v[:], scalar1=8.0, scalar2=4.0,
                op0=mybir.AluOpType.mult, op1=mybir.AluOpType.subtract,
            )
            e8 = sb.tile([128, 512], BF16, tag=f"te8{b}")
            nc.vector.tensor_tensor(
                out=e8[:], in0=t8[:], in1=sc[:], op=mybir.AluOpType.mult,
            )
            return chat, e8

        def table_product(b, chat, e8):
            tb = pst.tile([128, 512], F32, tag="tb")
            for c2 in range(2):
                cs = slice(256 * c2, 256 * c2 + 256)
                hi = slice(256 + 128 * c2, 384 + 128 * c2)
                nc.tensor.matmul(
                    out=tb[:, cs], lhsT=chat[:, hi], rhs=chat[:, 0:256],
                    start=True, stop=False,
                )
                nc.tensor.matmul(
                    out=tb[:, cs], lhsT=e8[:, hi], rhs=e8[:, 0:256],
                    start=False, stop=True,
                )
            return tb

        # ---- interleaved emission: x units flow while table DVE runs.
        # qx_unit(1) reuses the qz psum tile, so both table chains (whose
        # Sins free it) must be queued on ACT before sin(u1).
        hs_ps = psh.tile([16, 512], F32, tag="hs")
        ch0, e80 = table_chain(0)
        ch1, e81 = table_chain(1)
        s0, d0 = qx_unit(0)
        s1, d1 = qx_unit(1)
        tbs = [table_product(0, ch0, e80)]
        s2, d2 = qx_unit(2)
        tbs.append(table_product(1, ch1, e81))
        ds = [d0, d1, d2]
        s_last = s2
        for u in range(3, 8):
            s_last, du = qx_unit(u)
            ds.append(du)
            hfold(u - 3, ds[u - 3], hs_ps)
        for u in range(5, 8):
            hfold(u, ds[u], hs_ps)

        # ---- logZ. The Exp bias tile reads the LAST x sin so the
        # scheduler cannot hoist Exp (nl-exp set) between the Sins.
        cb = sb.tile([128, 1], F32, tag="cb")
        nc.scalar.activation(
            out=cb[:], in_=s_last[:, 0:1],
            func=mybir.ActivationFunctionType.Identity,
            scale=0.0, bias=bm60[:],
        )
        e_sb = sb.tile([128, 1024], BF16, tag="e")
        sums2 = sb.tile([128, 2], F32, tag="sums2")
        for b in range(BPC):
            nc.scalar.activation(
                out=e_sb[:, 512 * b : 512 * b + 512], in_=tbs[b][:],
                func=mybir.ActivationFunctionType.Exp,
                bias=cb[:],
                accum_out=sums2[:, b : b + 1],
            )
        small_ps = pss.tile([16, 1], F32, tag="small")
        nc.tensor.matmul(
            out=small_ps[0:2, 0:1], lhsT=sums2[:], rhs=ones[:],
            start=True, stop=True,
        )
        logz2 = sb.tile([2, 1], F32, tag="logz2")
        nc.scalar.activation(
            out=logz2[:], in_=small_ps[0:2, 0:1],
            func=mybir.ActivationFunctionType.Ln,
        )
        # broadcast -ln(S_b) to the 16 output partitions (reuses the bank)
        nz_ps = small_ps
        nc.tensor.matmul(
            out=nz_ps[:], lhsT=negsel[:], rhs=logz2[:], start=True, stop=True
        )
        # out = (128 + hs) - (60 + lnS) = hs + (68 - lnS)
        nz_sb = sb.tile([16, 1], F32, tag="nzsb")
        nc.vector.tensor_scalar(
            out=nz_sb[:], in0=nz_ps[:], scalar1=128.0 - EXP_SHIFT,
            scalar2=None, op0=mybir.AluOpType.add,
        )
        # final add on ACT (Identity + per-partition bias) — DVE is busy
        o_t = sb.tile([16, 512], F32, tag="o")
        nc.scalar.activation(
            out=o_t[:], in_=hs_ps[:],
            func=mybir.ActivationFunctionType.Identity,
            scale=1.0, bias=nz_sb[:],
        )
        nc.sync.dma_start(out=out[:], in_=o_t[:])

    nc.finalize()
    return nc


_NC = None


def _get_nc():
    global _NC
    if _NC is None:
        _NC = _build()
    return _NC


def _make_in_maps(x, W, r):
    x = np.asarray(x, dtype=np.int32)
    W = np.asarray(W, dtype=np.float32)
    r = np.asarray(r, dtype=np.float32)

    v = np.arange(256, dtype=np.int32)
    k8 = np.arange(8, dtype=np.int32)
    bp8 = ((v[None, :] >> k8[:, None]) & 1).astype(np.float32)  # [8, 256]
    bits9 = np.ones((9, 256), dtype=np.float32)
    bits9[0:8] = bp8
    bits9 = bits9.astype(ml_dtypes.bfloat16)

    k16 = np.arange(16, dtype=np.int32)
    aux = np.zeros((128, 256), dtype=np.float32)
    for vg in range(16):
        aux[:, 16 * vg + vg] = 8.0
    aux = aux.astype(ml_dtypes.bfloat16)
    negsel = np.zeros((2, 16), dtype=np.float32)
    negsel[0, 0:8] = -1.0
    negsel[1, 8:16] = -1.0

    in_maps = []
    for core in range(N_CORES):
        wmeta = np.zeros((17, 1536), dtype=ml_dtypes.bfloat16)
        wmeta[0:9, 1024:1280] = bits9
        bxs = []
        for b_loc in range(BPC):
            b = BPC * core + b_loc
            Wp = (W[b].T * INV_2PI).astype(ml_dtypes.bfloat16)  # [16, 128]
            rp = (r[b] * INV_2PI).astype(ml_dtypes.bfloat16).astype(np.float32)
            for half in range(2):
                # hi half negated: flips s and sc (making e8_hi = -8 e_hi)
                # but leaves v, chat unchanged
                sgn = -1.0 if half else 1.0
                cs = slice(128 * (2 * b_loc + half), 128 * (2 * b_loc + half) + 128)
                wmeta[0:8, cs] = sgn * Wp[8 * half : 8 * half + 8]
                wmeta[8, cs] = 0.0 if half else rp.astype(ml_dtypes.bfloat16)
            xs = slice(1280 + 128 * b_loc, 1280 + 128 * b_loc + 128)
            wmeta[0:16, xs] = Wp
            # frac-centered r' keeps |q| < 2 (the Chebyshev form is
            # invariant under integer shifts of q)
            rpf = rp - np.round(rp)
            wmeta[16, xs] = rpf.astype(ml_dtypes.bfloat16)
            bx = np.ones((17, 4096), dtype=np.float32)
            bx[0:16] = ((x[b][None, :] >> k16[:, None]) & 1).astype(np.float32)
            bxs.append(bx.astype(ml_dtypes.bfloat16))
        in_maps.append(
            {
                "wmeta": wmeta,
                "bitsx": np.concatenate(bxs, axis=1),
                "aux": aux,
                "negsel": negsel,
            }
        )
    return in_maps


def _run(x, W, r, trace=False):
    nc = _get_nc()
    in_maps = _make_in_maps(x, W, r)
    res = run_bass_kernel_spmd(nc, in_maps, core_ids=list(range(N_CORES)), trace=trace)
    outs = []
    for core in range(N_CORES):
        o = res.results[core]["out"].astype(np.float32)  # [16, 512]
        for b_loc in range(BPC):
            # row vg = 8b + 2u + h2 covers t = 1024u + 512h2 + j
            outs.append(o[8 * b_loc : 8 * b_loc + 8].reshape(4096))
    return np.stack(outs, axis=0), res


def kernel(x, W, r):
    out, _ = _run(x, W, r)
    return out


def kernel_traced(x, W, r):
    out, res = _run(x, W, r, trace=True)
    return out, res


# revision 34
# speedup vs baseline: 1.1027x; 1.1027x over previous
"""BIDE forward kernel for Trainium2, 8-core data parallel over B — v5.2.

Math (per batch row; 2 rows per core):
  logit(v) = sum_h cos(2 pi z[h,v]),  z = W' . bits(v) + r'  (W' = W/2pi)
  out[t] = logit(x_t) - logZ,  logZ = 60 + ln sum_v exp(logit(v) - 60)

All trig goes through ONE spline (Sin, domain [-pi, pi]) via the
Chebyshev double-double-angle form
    cos(2 pi q) = 1 + 8 d,   d = (v - 1) v,   v = sin^2(pi q / 2)
exact for |q| <= 2 and invariant under integer shifts of q.

logZ path: z splits into zlo (low 8 bits, + r') and zhi (high 8 bits),
each enumerated over 256 values; |z| <= 0.93 < 1 (verified from W), so
s = sin(pi z/2) and c = sin(pi z/2 + pi/2) need NO range reduction.
DVE builds chat = cos(2 pi z) = 1+8d and e8 = (8v-4)sc = 8(v-0.5)sc
(hi-half weights negated on host, so e8_hi = -8 e_hi while chat is
unaffected); the 256x256 logit table is then just two K=128 matmuls
per 128-row half:
    table = chat_hi^T chat_lo + e8_hi^T e8_lo
exp(table - 60) with accum_out gives S_b per partition.

logit_x path: q[h,t] = W' bits(x_t) + frac(r') via a K=17 matmul
(|q| <= 1.27 for every attainable pattern), Sin straight from PSUM,
v = s*s, vm1 = v-1, d = v*vm1 on DVE, and an h-fold matmul whose
one-hot weights carry the 8x: hs = 8 sum_h d.  out = hs + (68 - lnS).

Scheduling (all HW-measured this session):
 - PE runs at 1.2 GHz on this instance (a 3.6us back-to-back matmul
   warmup never lifted the HAM clock gate), so matmul columns are the
   budget; the table product is reduced to 2 MMs/half and emission
   interleaves x-path matmuls between table groups to avoid PE stalls.
 - ACT queue must be [all Sins][Exp Exp][Ln] for minimal table-set
   loads; the Exp bias tile is produced by an ACT Identity that READS
   the last x-sin output, so the scheduler topologically cannot hoist
   Exp between Sins. get_activation_tables is patched so Exp and Ln
   share natural_log_exp_and_others (one load).
 - The final output add runs on ACT (Identity + per-partition bias),
   off the busy DVE.
 - PSUM (8 banks): qz+x-unit pool 2x[128,1024] (4) + table products 2
   + h-fold accumulator 1 + logZ scalars 1.

Dead ends (HW-measured): indirect-DMA gathers ~4.5ns/desc serialized;
gpsimd ap_gather ~27us/512; GPSIMD tensor_scalar 7-15us AND stalls
concurrent DVE ops; GPSIMD cannot read PSUM, no tensor_tensor codegen;
AluOpType.mod rejected; ACT Sin garbage outside [-pi,pi]; re-opening a
closed PSUM accumulation group overwrites instead of accumulating;
scalar_tensor_tensor runs at 1x on DVE.
"""

import numpy as np
import ml_dtypes
from contextlib import ExitStack

import concourse.bacc as bacc
import concourse.bass as bass
from concourse import mybir
from concourse.bass_utils import run_bass_kernel_spmd
from concourse.tile import TileContext

F32 = mybir.dt.float32
BF16 = mybir.dt.bfloat16

PI_2 = float(np.float32(np.pi / 2.0))
INV_2PI = 1.0 / (2.0 * np.pi)
# logits peak ~89: shift exp so it stays in fp32 / the ACT Ln spline range
EXP_SHIFT = 60.0

N_CORES = 8
B, H, T = 16, 128, 4096
BPC = B // N_CORES  # batch rows per core (2)


def _patch_act_tables(arch: str):
    """Force Exp and Ln to resolve to natural_log_exp_and_others so the
    [sins][exp][ln] queue needs one nl-exp set load, not two."""
    from concourse.hw_specs import get_activation_tables

    tabs = get_activation_tables(arch)
    for name, fns in tabs.items():
        if name != "natural_log_exp_and_others":
            fns.discard(mybir.ActivationFunctionType.Exp)
            fns.discard(mybir.ActivationFunctionType.Ln)


def _build():
    nc = bacc.Bacc("TRN2", target_bir_lowering=False, debug=False)
    _patch_act_tables(nc.m.arch)

    # packed weights: cols [0:512) table z blocks [9,128] at 128*(2b+half)
    # (rows 0-7 bit weights — hi half negated; row 8 = r' on lo, 0 on hi),
    # [1024:1280) bit-plane enumeration bits9, [1280:1536) wx (x-path;
    # rows 0-15 W'.T, 16 frac(r'); block b at 1280+128b)
    wmeta = nc.dram_tensor("wmeta", [17, 1536], BF16, kind="ExternalInput")
    # bit-planes of x: row n = bit_n(x[b, t]), row 16 = 1; col 4096b + t
    bitsx = nc.dram_tensor("bitsx", [17, 8192], BF16, kind="ExternalInput")
    # h-sum one-hot columns scaled by 8: aux[h, 16vg+m] = 8*(m==vg)
    aux = nc.dram_tensor("aux", [128, 256], BF16, kind="ExternalInput")
    # negsel[k, m] = -1 if m//8 == k else 0 (broadcasts -ln S_b)
    negsel_in = nc.dram_tensor("negsel", [2, 16], F32, kind="ExternalInput")
    # row vg = 8b + 2u + h2 covers t in [1024u + 512h2, +512) of batch b
    out = nc.dram_tensor("out", [16, 512], F32, kind="ExternalOutput")

    with ExitStack() as ctx:
        tc = ctx.enter_context(TileContext(nc))
        sb = ctx.enter_context(tc.tile_pool(name="sb", bufs=1))
        px = ctx.enter_context(tc.tile_pool(name="px", bufs=2, space="PSUM"))
        pst = ctx.enter_context(tc.tile_pool(name="pst", bufs=2, space="PSUM"))
        psh = ctx.enter_context(tc.tile_pool(name="psh", bufs=1, space="PSUM"))
        pss = ctx.enter_context(tc.tile_pool(name="pss", bufs=1, space="PSUM"))

        # ---- input loads on two parallel HWDGE queues
        wmeta_sb = sb.tile([17, 1536], BF16, tag="wmeta")
        bitsx_sb = sb.tile([17, 8192], BF16, tag="bitsx")
        aux_sb = sb.tile([128, 256], BF16, tag="aux")
        negsel = sb.tile([2, 16], F32, tag="negsel")
        nc.sync.dma_start(out=wmeta_sb[:], in_=wmeta[:])
        nc.sync.dma_start(out=bitsx_sb[:, 0:4096], in_=bitsx[:, 0:4096])
        nc.sync.dma_start(out=bitsx_sb[:, 4096:8192], in_=bitsx[:, 4096:8192])
        nc.scalar.dma_start(out=aux_sb[:], in_=aux[:])
        nc.scalar.dma_start(out=negsel[:], in_=negsel_in[:])

        # ---- constants
        ones = sb.tile([128, 1], F32, tag="ones")
        nc.vector.memset(ones[:], 1.0)
        bpi2 = sb.tile([128, 1], F32, tag="bpi2")
        nc.vector.memset(bpi2[:], PI_2)
        bm60 = sb.tile([128, 1], F32, tag="bm60")
        nc.vector.memset(bm60[:], -EXP_SHIFT)

        # ---- table path: z enumerations in pst-pool tiles (the tb slots
        # are free this early), keeping the px pool exclusively for x
        # units so u1 need not wait for the table sins
        qzs = []
        for b in range(BPC):
            qzb = pst.tile([128, 512], F32, tag="tb")
            qzs.append(qzb)
            for half in range(2):
                cs = 128 * (2 * b + half)
                nc.tensor.matmul(
                    out=qzb[:, 256 * half : 256 * half + 256],
                    lhsT=wmeta_sb[0:9, cs : cs + 128],
                    rhs=wmeta_sb[0:9, 1024:1280],
                    start=True, stop=True,
                )

        # ---- x path part 1: first two units' matmuls (emitted early so
        # the PE queue has work while the table's DVE chain runs)
        def qx_unit(u):
            b, cu = divmod(u, 4)
            col = 4096 * b + 1024 * cu
            qx = px.tile([128, 1024], F32, tag="u")
            for h2 in range(2):
                nc.tensor.matmul(
                    out=qx[:, 512 * h2 : 512 * h2 + 512],
                    lhsT=wmeta_sb[0:17, 1280 + 128 * b : 1408 + 128 * b],
                    rhs=bitsx_sb[0:17, col + 512 * h2 : col + 512 * h2 + 512],
                    start=True, stop=True,
                )
            s = sb.tile([128, 1024], BF16, tag=f"s{u}")
            nc.scalar.activation(
                out=s[:], in_=qx[:],
                func=mybir.ActivationFunctionType.Sin, scale=PI_2,
            )
            v = sb.tile([128, 1024], BF16, tag=f"v{u}")
            nc.vector.tensor_tensor(
                out=v[:], in0=s[:], in1=s[:], op=mybir.AluOpType.mult,
            )
            vm1 = sb.tile([128, 1024], BF16, tag=f"vm{u}")
            nc.vector.tensor_scalar(
                out=vm1[:], in0=v[:], scalar1=1.0, scalar2=None,
                op0=mybir.AluOpType.subtract,
            )
            d = sb.tile([128, 1024], BF16, tag=f"d{u}")
            nc.vector.tensor_tensor(
                out=d[:], in0=v[:], in1=vm1[:], op=mybir.AluOpType.mult,
            )
            return s, d

        def hfold(u, d, hs_ps):
            for h2 in range(2):
                vg = 2 * u + h2
                nc.tensor.matmul(
                    out=hs_ps[:],
                    lhsT=aux_sb[:, 16 * vg : 16 * vg + 16],
                    rhs=d[:, 512 * h2 : 512 * h2 + 512],
                    start=(vg == 0), stop=(vg == 15),
                )

        # table trig + DVE chain per b -> chat (=cos 2 pi z) and e8
        def table_chain(b):
            s = sb.tile([128, 512], BF16, tag=f"ts{b}")
            nc.scalar.activation(
                out=s[:], in_=qzs[b][:],
                func=mybir.ActivationFunctionType.Sin, scale=PI_2,
            )
            c = sb.tile([128, 512], BF16, tag=f"tc{b}")
            nc.scalar.activation(
                out=c[:], in_=qzs[b][:],
                func=mybir.ActivationFunctionType.Sin, scale=PI_2,
                bias=bpi2[:],
            )
            v = sb.tile([128, 512], BF16, tag=f"tv{b}")
            nc.vector.tensor_tensor(
                out=v[:], in0=s[:], in1=s[:], op=mybir.AluOpType.mult,
            )
            sc = sb.tile([128, 512], BF16, tag=f"tsc{b}")
            nc.vector.tensor_tensor(
                out=sc[:], in0=s[:], in1=c[:], op=mybir.AluOpType.mult,
            )
            # chat = (8v-8)v + 1 = cos(2 pi z); e8 = (8v-4)sc
            v8m8 = sb.tile([128, 512], BF16, tag=f"t88{b}")
            nc.vector.tensor_scalar(
                out=v8m8[:], in0=v[:], scalar1=8.0, scalar2=8.0,
                op0=mybir.AluOpType.mult, op1=mybir.AluOpType.subtract,
            )
            cpre = sb.tile([128, 512], BF16, tag=f"tcp{b}")
            nc.vector.tensor_tensor(
                out=cpre[:], in0=v8m8[:], in1=v[:], op=mybir.AluOpType.mult,
            )
            chat = sb.tile([128, 512], BF16, tag=f"tch{b}")
            nc.vector.tensor_scalar(
                out=chat[:], in0=cpre[:], scalar1=1.0, scalar2=None,
                op0=mybir.AluOpType.add,
            )
            t8 = sb.tile([128, 512], BF16, tag=f"tt8{b}")
            nc.vector.tensor_scalar(
                out=t8[:], in0=v[:], scalar1=8.0, scalar2=4.0,
                op0=mybir.AluOpType.mult, op1=mybir.AluOpType.subtract,
            )
            e8 = sb.tile([128, 512], BF16, tag=f"te8{b}")
            nc.vector.tensor_tensor(
                out=e8[:], in0=t8[:], in1=sc[:], op=mybir.AluOpType.mult,
            )
            return chat, e8

        def table_product(b, chat, e8):
            tb = pst.tile([128, 512], F32, tag="tb")
            for c2 in range(2):
                cs = slice(256 * c2, 256 * c2 + 256)
                hi = slice(256 + 128 * c2, 384 + 128 * c2)
                nc.tensor.matmul(
                    out=tb[:, cs], lhsT=chat[:, hi], rhs=chat[:, 0:256],
                    start=True, stop=False,
                )
                nc.tensor.matmul(
                    out=tb[:, cs], lhsT=e8[:, hi], rhs=e8[:, 0:256],
                    start=False, stop=True,
                )
            return tb

        # ---- interleaved emission: x units flow while table DVE runs.
        # qx_unit(1) reuses the qz psum tile, so both table chains (whose
        # Sins free it) must be queued on ACT before sin(u1).
        hs_ps = psh.tile([16, 512], F32, tag="hs")
        ch0, e80 = table_chain(0)
        ch1, e81 = table_chain(1)
        s0, d0 = qx_unit(0)
        s1, d1 = qx_unit(1)
        tbs = [table_product(0, ch0, e80)]
        s2, d2 = qx_unit(2)
        tbs.append(table_product(1, ch1, e81))
        ds = [d0, d1, d2]
        s_last = s2
        for u in range(3, 8):
            s_last, du = qx_unit(u)
            ds.append(du)
            hfold(u - 3, ds[u - 3], hs_ps)
        for u in range(5, 8):
            hfold(u, ds[u], hs_ps)

        # ---- logZ. The Exp bias tile reads the LAST x sin so the
        # scheduler cannot hoist Exp (nl-exp set) between the Sins.
        cb = sb.tile([128, 1], F32, tag="cb")
        nc.scalar.activation(
            out=cb[:], in_=s_last[:, 0:1],
            func=mybir.ActivationFunctionType.Identity,
            scale=0.0, bias=bm60[:],
        )
        e_sb = sb.tile([128, 1024], BF16, tag="e")
        sums2 = sb.tile([128, 2], F32, tag="sums2")
        for b in range(BPC):
            nc.scalar.activation(
                out=e_sb[:, 512 * b : 512 * b + 512], in_=tbs[b][:],
                func=mybir.ActivationFunctionType.Exp,
                bias=cb[:],
                accum_out=sums2[:, b : b + 1],
            )
        small_ps = pss.tile([16, 1], F32, tag="small")
        nc.tensor.matmul(
            out=small_ps[0:2, 0:1], lhsT=sums2[:], rhs=ones[:],
            start=True, stop=True,
        )
        logz2 = sb.tile([2, 1], F32, tag="logz2")
        nc.scalar.activation(
            out=logz2[:], in_=small_ps[0:2, 0:1],
            func=mybir.ActivationFunctionType.Ln,
        )
        # broadcast -ln(S_b) to the 16 output partitions (reuses the bank)
        nz_ps = small_ps
        nc.tensor.matmul(
            out=nz_ps[:], lhsT=negsel[:], rhs=logz2[:], start=True, stop=True
        )
        # out = (128 + hs) - (60 + lnS) = hs + (68 - lnS)
        nz_sb = sb.tile([16, 1], F32, tag="nzsb")
        nc.vector.tensor_scalar(
            out=nz_sb[:], in0=nz_ps[:], scalar1=128.0 - EXP_SHIFT,
            scalar2=None, op0=mybir.AluOpType.add,
        )
        # final add on ACT (Identity + per-partition bias) — DVE is busy
        o_t = sb.tile([16, 512], F32, tag="o")
        nc.scalar.activation(
            out=o_t[:], in_=hs_ps[:],
            func=mybir.ActivationFunctionType.Identity,
            scale=1.0, bias=nz_sb[:],
        )
        nc.sync.dma_start(out=out[:], in_=o_t[:])

    nc.finalize()
    return nc


_NC = None


def _get_nc():
    global _NC
    if _NC is None:
        _NC = _build()
    return _NC


def _make_in_maps(x, W, r):
    x = np.asarray(x, dtype=np.int32)
    W = np.asarray(W, dtype=np.float32)
    r = np.asarray(r, dtype=np.float32)

    v = np.arange(256, dtype=np.int32)
    k8 = np.arange(8, dtype=np.int32)
    bp8 = ((v[None, :] >> k8[:, None]) & 1).astype(np.float32)  # [8, 256]
    bits9 = np.ones((9, 256), dtype=np.float32)
    bits9[0:8] = bp8
    bits9 = bits9.astype(ml_dtypes.bfloat16)

    k16 = np.arange(16, dtype=np.int32)
    aux = np.zeros((128, 256), dtype=np.float32)
    for vg in range(16):
        aux[:, 16 * vg + vg] = 8.0
    aux = aux.astype(ml_dtypes.bfloat16)
    negsel = np.zeros((2, 16), dtype=np.float32)
    negsel[0, 0:8] = -1.0
    negsel[1, 8:16] = -1.0

    in_maps = []
    for core in range(N_CORES):
        wmeta = np.zeros((17, 1536), dtype=ml_dtypes.bfloat16)
        wmeta[0:9, 1024:1280] = bits9
        bxs = []
        for b_loc in range(BPC):
            b = BPC * core + b_loc
            Wp = (W[b].T * INV_2PI).astype(ml_dtypes.bfloat16)  # [16, 128]
            rp = (r[b] * INV_2PI).astype(ml_dtypes.bfloat16).astype(np.float32)
            for half in range(2):
                # hi half negated: flips s and sc (making e8_hi = -8 e_hi)
                # but leaves v, chat unchanged
                sgn = -1.0 if half else 1.0
                cs = slice(128 * (2 * b_loc + half), 128 * (2 * b_loc + half) + 128)
                wmeta[0:8, cs] = sgn * Wp[8 * half : 8 * half + 8]
                wmeta[8, cs] = 0.0 if half else rp.astype(ml_dtypes.bfloat16)
            xs = slice(1280 + 128 * b_loc, 1280 + 128 * b_loc + 128)
            wmeta[0:16, xs] = Wp
            # frac-centered r' keeps |q| < 2 (the Chebyshev form is
            # invariant under integer shifts of q)
            rpf = rp - np.round(rp)
            wmeta[16, xs] = rpf.astype(ml_dtypes.bfloat16)
            bx = np.ones((17, 4096), dtype=np.float32)
            bx[0:16] = ((x[b][None, :] >> k16[:, None]) & 1).astype(np.float32)
            bxs.append(bx.astype(ml_dtypes.bfloat16))
        in_maps.append(
            {
                "wmeta": wmeta,
                "bitsx": np.concatenate(bxs, axis=1),
                "aux": aux,
                "negsel": negsel,
            }
        )
    return in_maps


def _run(x, W, r, trace=False):
    nc = _get_nc()
    in_maps = _make_in_maps(x, W, r)
    res = run_bass_kernel_spmd(nc, in_maps, core_ids=list(range(N_CORES)), trace=trace)
    outs = []
    for core in range(N_CORES):
        o = res.results[core]["out"].astype(np.float32)  # [16, 512]
        for b_loc in range(BPC):
            # row vg = 8b + 2u + h2 covers t = 1024u + 512h2 + j
            outs.append(o[8 * b_loc : 8 * b_loc + 8].reshape(4096))
    return np.stack(outs, axis=0), res


def kernel(x, W, r):
    out, _ = _run(x, W, r)
    return out


def kernel_traced(x, W, r):
    out, res = _run(x, W, r, trace=True)
    return out, res
